# revision 1
# baseline (speedup 1.0000x reference)
"""LinearSelfAttention kernel for TRN2 (8 NeuronCores, batch-parallel).

Key identity: with Hn = H[:, :n] (mask drops column n from the s-sum),
    attn = P H mask(H^T Q H) = C H,   C = P G Q,   G = Hn Hn^T  (257x257)
so  out = H + C H / n = Et^T H,  Et = I + Q^T G P^T / n.
O(n d^2) for G and Et^T H plus O(d^3) for the tiny chain, vs O(3 n d^2)
for the naive re-association.

PE strategy:
 - G via fp8e4m3 DoubleRow (K=256/pass, 0.5 cyc/row), host-transposed Hn.
 - Final Et^T H in bf16 with the +H identity folded into Et (FWL keeps
   LDWEIGHTS hidden; fp8 DoubleRow loses FWL and is a net loss here).
 - The K=1 matmuls from the e=256 row are packed into disjoint 32-row
   PE strips (tile_position) so they pipeline back-to-back.
 - Output row d=256 (the M=1 edge) is computed exactly on the host.
 - DMAs batched aggressively: each DMA_DIRECT2D costs ~0.6us of engine
   issue time, so fewer/bigger transfers win.

Sharding: data-parallel over batch, 2 samples per core.
"""

import sys

sys.path.insert(0, "/opt/trn_rl_repo")

import numpy as np
import ml_dtypes

B, D1, N1 = 16, 257, 2049  # batch, d+1, n+1
N = N1 - 1  # 2048
NCORES = 8
BPC = B // NCORES  # samples per core

# partition chunking of the 257-sized dims: (offset, size)
CH = [(0, 128), (128, 128), (256, 1)]
NT8 = N // 256  # 8 double-row s-tiles of the transposed Hn
DPAD = 272  # fp8 DR LDWEIGHTS: step between the 2 K-subtiles must be %16==0
TCH = [(i * 512, 512) for i in range(4)]  # t=2048 column done on host
NWARM = 22

_cached = {}


def _build():
    import concourse.bass as bass
    import concourse.tile as tile
    from concourse import bacc, mybir
    from contextlib import ExitStack

    f32 = mybir.dt.float32
    bf16 = mybir.dt.bfloat16
    f8 = mybir.dt.float8e4
    DR = mybir.MatmulPerfMode.DoubleRow

    nc = bacc.Bacc("TRN2", target_bir_lowering=False, debug=False, num_devices=NCORES)

    Hb_d = nc.declare_dram_parameter("Hb", [BPC, D1, N1], bf16, isOutput=False)
    Ht_d = nc.declare_dram_parameter("Ht", [BPC, NT8, 128, 2, DPAD], f8, isOutput=False)
    QPI_d = nc.declare_dram_parameter("QPI", [D1, 3 * D1], bf16, isOutput=False)
    E3_d = nc.declare_dram_parameter("E3", [BPC, 3 * D1], bf16, isOutput=False)
    Y_d = nc.declare_dram_parameter("Y", [BPC, 256, N], bf16, isOutput=True)

    with tile.TileContext(nc) as tc:
        with ExitStack() as ctx:
            const = ctx.enter_context(tc.tile_pool(name="const", bufs=1))
            htp = ctx.enter_context(tc.tile_pool(name="htp", bufs=2))
            hbp = ctx.enter_context(tc.tile_pool(name="hbp", bufs=2))
            sq = ctx.enter_context(tc.tile_pool(name="sq", bufs=2))
            yp = ctx.enter_context(tc.tile_pool(name="yp", bufs=2))

            # ---- input DMAs, batched; per-queue FIFO keeps sample 0 first
            ht = [None] * BPC
            hb = [None] * BPC
            hr = [None] * BPC
            e3t = [None] * BPC
            qpi = []
            for b in range(BPC):
                # 8 DR s-tiles of Hn^T, split across three queues; both
                # samples' G operands load before anything else
                t = htp.tile([128, NT8, 2, DPAD], f8, tag="ht", name=f"ht{b}")
                if b == 0:
                    nc.sync.dma_start(t[:, 0:3, :, :], Ht_d[b, 0:3])
                    nc.scalar.dma_start(t[:, 3:6, :, :], Ht_d[b, 3:6])
                    nc.gpsimd.dma_start(t[:, 6:8, :, :], Ht_d[b, 6:8])
                else:
                    nc.sync.dma_start(t[:, 0:4, :, :], Ht_d[b, 0:4])
                    nc.scalar.dma_start(t[:, 4:6, :, :], Ht_d[b, 4:6])
                    nc.gpsimd.dma_start(t[:, 6:8, :, :], Ht_d[b, 6:8])
                ht[b] = t
            for b in range(BPC):
                # host-computed 257-edge vectors [G[256,:] | V[256,:] | Ct[256,:]]
                t = hbp.tile([128, 3 * D1], bf16, tag="e3", name=f"e3_{b}")
                nc.sync.dma_start(t[0:1, :], E3_d[b])
                nc.sync.dma_start(t[32:33, :], E3_d[b])
                e3t[b] = t
            qq = [nc.sync, nc.scalar, nc.gpsimd]
            for c, (off, sz) in enumerate(CH):
                t = const.tile([128, 3 * D1], bf16, tag=f"qpi{c}", name=f"qpi{c}")
                qq[c].dma_start(t[:sz, :], QPI_d[off : off + sz, :])
                if c == 2:  # row 256 also at partition 32 for packed K=1s
                    nc.gpsimd.dma_start(t[32:33, :], QPI_d[256:257, :])
                qpi.append(t)
            for b in range(BPC):
                # H rows 0..255: [p, c, t] = H[c*128+p, t]
                t = hbp.tile([128, 2, N1], bf16, tag="hb", name=f"hb{b}")
                nc.scalar.dma_start(t[:, 0, :], Hb_d[b, 0:128, :])
                nc.scalar.dma_start(t[:, 1, :], Hb_d[b, 128:256, :])
                hb[b] = t
                # H row e=256 replicated at partitions 0/32 for the edges
                t = hbp.tile([128, N1], bf16, tag="hr", name=f"hr{b}")
                nc.gpsimd.dma_start(t[0:1, :], Hb_d[b, 256:257, :])
                nc.sync.dma_start(t[32:33, :], Hb_d[b, 256:257, :])
                hr[b] = t


            # ---- PE warmup: ride the clock ramp until the first tile lands
            wsb = const.tile([128, 128], bf16, tag="wsb", name="wsb")
            nc.vector.memset(wsb[:, :], 0.0)
            with tc.tile_pool(name="wp", bufs=1, space="PSUM") as wp:
                wps = wp.tile([128, 512], f32, tag="wps", name="warm_ps")
                for i in range(NWARM):
                    nc.tensor.matmul(
                        wps[:, 0:128],
                        wsb[:, :],
                        wsb[:, :],
                        start=(i == 0),
                        stop=(i == NWARM - 1),
                    )

            with (
                tc.tile_pool(name="ppa", bufs=4, space="PSUM") as ppa,
                tc.tile_pool(name="ppb", bufs=4, space="PSUM") as ppb,
            ):
                # [128,512] banks from tags A/B, both 4-deep; all 257-edge
                # vectors arrive precomputed from the host (E3)
                gsb = [None] * BPC
                vsb = [None] * BPC
                et = [None] * BPC

                def g_stage(b):
                    # ---- G = Hn Hn^T (fp8 DoubleRow, K=256 per pass)
                    gA = ppa.tile([128, 512], f32, tag="A", name=f"gA{b}")
                    gB = ppb.tile([128, 512], f32, tag="B", name=f"gB{b}")
                    regions = [gA[:, 0:D1], gB[:, 0:D1]]
                    for st in range(NT8):
                        for ac in range(2):
                            nc.tensor.matmul(
                                regions[ac][:, :],
                                ht[b][:, st, :, ac * 128 : (ac + 1) * 128],
                                ht[b][:, st, :, :D1],
                                start=(st == 0),
                                stop=(st == NT8 - 1),
                                perf_mode=DR,
                            )
                    gs = []
                    for ac in range(2):
                        t = sq.tile([128, D1], bf16, tag=f"g{ac}", name=f"gs{b}_{ac}")
                        eng = nc.scalar.copy if ac % 2 == 0 else nc.vector.tensor_copy
                        eng(t[:, :], regions[ac][:, :])
                        gs.append(t)
                    gsb[b] = gs

                def vct_stage(b):
                    # ---- V = G P^T  (G symmetric: lhsT slices G directly)
                    vA = ppa.tile([128, 512], f32, tag="A", name=f"vA{b}")
                    vB = ppb.tile([128, 512], f32, tag="B", name=f"vB{b}")
                    regions = [vA[:, 0:D1], vB[:, 0:D1]]
                    for kb in range(2):
                        for am in range(2):
                            nc.tensor.matmul(
                                regions[am][:, :],
                                gsb[b][kb][:, am * 128 : (am + 1) * 128],
                                qpi[kb][:, D1 : 2 * D1],
                                start=(kb == 0),
                                stop=False,
                            )
                    nc.tensor.matmul(
                        regions[0][:, :],
                        e3t[b][0:1, 0:128],
                        qpi[2][0:1, D1 : 2 * D1],
                        start=False,
                        stop=True,
                    )
                    nc.tensor.matmul(
                        regions[1][:, :],
                        e3t[b][32:33, 128:256],
                        qpi[2][32:33, D1 : 2 * D1],
                        start=False,
                        stop=True,
                    )
                    vs = []
                    for am in range(2):
                        t = sq.tile([128, D1], bf16, tag=f"v{am}", name=f"vs{b}_{am}")
                        eng = nc.scalar.copy if am % 2 == 1 else nc.vector.tensor_copy
                        eng(t[:, :], regions[am][:, :])
                        vs.append(t)
                    vsb[b] = vs

                    # ---- Ct = (Q/n)^T V  (= C^T/n), then Et = I + Ct
                    cA = ppa.tile([128, 512], f32, tag="A", name=f"cA{b}")
                    cB = ppb.tile([128, 512], f32, tag="B", name=f"cB{b}")
                    cregions = [cA[:, 0:D1], cB[:, 0:D1]]
                    for ka in range(2):
                        for em in range(2):
                            nc.tensor.matmul(
                                cregions[em][:, :],
                                qpi[ka][:, em * 128 : (em + 1) * 128],
                                vsb[b][ka][:, :],
                                start=(ka == 0),
                                stop=False,
                            )
                    nc.tensor.matmul(
                        cregions[0][:, :],
                        qpi[2][0:1, 0:128],
                        e3t[b][0:1, D1 : 2 * D1],
                        start=False,
                        stop=True,
                    )
                    nc.tensor.matmul(
                        cregions[1][:, :],
                        qpi[2][32:33, 128:256],
                        e3t[b][32:33, D1 : 2 * D1],
                        start=False,
                        stop=True,
                    )
                    ets = []
                    for em in range(2):
                        t = sq.tile([128, D1], bf16, tag=f"e{em}", name=f"et{b}_{em}")
                        nc.vector.tensor_add(
                            t[:, :],
                            cregions[em][:, :],
                            qpi[em][:, 2 * D1 : 3 * D1],
                        )
                        ets.append(t)
                    et[b] = ets

                def s4_stage(b):
                    # ---- Y[d<256] = Et^T H
                    y = [
                        yp.tile([128, N], bf16, tag=f"y{dc}", name=f"y{b}_{dc}")
                        for dc in range(2)
                    ]
                    for ti, (toff, tsz) in enumerate(TCH):
                        p0 = ppa.tile([128, 512], f32, tag="A", name=f"pa{b}_{ti}")
                        p1 = ppb.tile([128, 512], f32, tag="B", name=f"pb{b}_{ti}")
                        for dc, p in ((0, p0), (1, p1)):
                            dsl = slice(dc * 128, (dc + 1) * 128)
                            for ec in range(2):
                                nc.tensor.matmul(
                                    p[:128, :tsz],
                                    et[b][ec][:, dsl],
                                    hb[b][:, ec, toff : toff + tsz],
                                    start=(ec == 0),
                                    stop=False,
                                )
                        # K=1 edges, packed into PE row strips 0/32
                        nc.tensor.matmul(
                            p0[:128, :tsz],
                            e3t[b][0:1, 2 * D1 : 2 * D1 + 128],
                            hr[b][0:1, toff : toff + tsz],
                            start=False,
                            stop=True,
                        )
                        nc.tensor.matmul(
                            p1[:128, :tsz],
                            e3t[b][32:33, 2 * D1 + 128 : 2 * D1 + 256],
                            hr[b][32:33, toff : toff + tsz],
                            start=False,
                            stop=True,
                        )
                        if b == BPC - 1 and ti == 3:
                            # final chunk: split evictions across both engines
                            # and store each 65KB half as it lands, so the
                            # post-compute drain is as short as possible
                            nc.scalar.copy(y[0][:, 1536:1792], p0[:128, 0:256])
                            nc.vector.tensor_copy(
                                y[1][:, 1536:1792], p1[:128, 0:256]
                            )
                            nc.sync.dma_start(
                                Y_d[b, 0:128, 1536:1792], y[0][:, 1536:1792]
                            )
                            nc.gpsimd.dma_start(
                                Y_d[b, 128:256, 1536:1792], y[1][:, 1536:1792]
                            )
                            nc.scalar.copy(y[0][:, 1792:N], p0[:128, 256:512])
                            nc.vector.tensor_copy(y[1][:, 1792:N], p1[:128, 256:512])
                            nc.sync.dma_start(
                                Y_d[b, 0:128, 1792:N], y[0][:, 1792:N]
                            )
                            nc.gpsimd.dma_start(
                                Y_d[b, 128:256, 1792:N], y[1][:, 1792:N]
                            )
                        else:
                            nc.scalar.copy(y[0][:, toff : toff + tsz], p0[:128, :tsz])
                            nc.vector.tensor_copy(
                                y[1][:, toff : toff + tsz], p1[:128, :tsz]
                            )
                        # store as evictions complete; both samples stagger
                        # per-chunk so the queues stay drained and only 65KB
                        # halves remain after the final eviction
                        if ti in (1, 2):
                            sl = slice(0, 1024) if ti == 1 else slice(1024, 1536)
                            nc.sync.dma_start(Y_d[b, 0:128, sl], y[0][:, sl])
                            nc.gpsimd.dma_start(Y_d[b, 128:256, sl], y[1][:, sl])
                    if b < BPC - 1:
                        nc.sync.dma_start(Y_d[b, 0:128, 1536:N], y[0][:, 1536:N])
                        nc.gpsimd.dma_start(Y_d[b, 128:256, 1536:N], y[1][:, 1536:N])


                # emission order fills the small-chain eviction stalls with
                # the other sample's G, then runs both S4s as one dense block
                g_stage(0)
                g_stage(1)
                vct_stage(0)
                vct_stage(1)
                s4_stage(0)
                s4_stage(1)

    nc.compile()
    return nc


def _prep_in_maps(H, P, Q):
    bf = ml_dtypes.bfloat16
    f8 = ml_dtypes.float8_e4m3
    H = np.ascontiguousarray(H, dtype=np.float32)
    Hb = H.astype(bf)
    # G operand: [st, p, i, d] with s = st*256 + i*128 + p
    Ht = np.swapaxes(H[:, :, :N], 1, 2).reshape(B, NT8, 2, 128, D1)
    Ht8 = np.zeros((B, NT8, 128, 2, DPAD), dtype=f8)
    Ht8[..., :D1] = np.swapaxes(Ht, 2, 3).astype(f8)
    QPI = np.concatenate(
        [Q / N, P.T, np.eye(D1, dtype=np.float32)], axis=1
    ).astype(bf)
    QPI = np.ascontiguousarray(QPI)
    # exact 257-edge vectors of the G/V/Ct chain, via Hn matvecs
    Hn = H[:, :, :N]
    g256 = np.einsum("bds,bs->bd", Hn, H[:, 256, :N])  # G[:,256] = G[256,:]
    v256 = g256 @ P.T  # V[256,:]
    t1 = np.einsum("bds,d->bs", Hn, np.ascontiguousarray(Q[:, 256], np.float32))
    r = np.einsum("bds,bs->bd", Hn, t1)  # G @ Q[:,256]
    er256 = (r @ P.T) / N  # Ct[256,:]
    E3 = np.concatenate([g256, v256, er256], axis=1).astype(bf)
    return [
        {
            "Hb": Hb[c * BPC : (c + 1) * BPC],
            "Ht": Ht8[c * BPC : (c + 1) * BPC],
            "QPI": QPI,
            "E3": E3[c * BPC : (c + 1) * BPC],
        }
        for c in range(NCORES)
    ]


def kernel(H, P, Q):
    from concourse.bass_utils import run_bass_kernel_spmd

    if "nc" not in _cached:
        _cached["nc"] = _build()
    nc = _cached["nc"]

    in_maps = _prep_in_maps(H, P, Q)
    for attempt in range(3):
        res = run_bass_kernel_spmd(nc, in_maps, list(range(NCORES)))
        ydev = np.concatenate(
            [res.results[c]["Y"].astype(np.float32) for c in range(NCORES)], axis=0
        )
        if np.isfinite(ydev).all():
            break
    out = np.empty((B, D1, N1), dtype=np.float32)
    out[:, :256, :N] = ydev
    # output row d=256 exactly, on host (fp32): avoids the M=1 PE chunks
    H = np.ascontiguousarray(H, dtype=np.float32)
    Hn = H[:, :, :N]
    u = np.einsum("bds,d->bs", Hn, np.ascontiguousarray(P[256, :], np.float32))
    v = np.einsum("bds,bs->bd", Hn, u)  # = G @ P[256,:] per sample
    c256 = v @ Q  # = C[256, :] per sample
    out[:, 256, :] = H[:, 256, :] + np.einsum("bd,bdt->bt", c256, H) / N
    # exact column t=2048 on host: C @ hcol = P (G (Q hcol))
    hcol = H[:, :, N]  # (B, 257)
    w1 = hcol @ Q.T  # wait: (Q hcol)[a] = sum_e Q[a,e] hcol[e]
    w2 = np.einsum("bds,bs->bd", Hn, np.einsum("bds,bd->bs", Hn, w1))  # G w1
    w3 = w2 @ P.T  # (P w2)[d]
    out[:, :256, N] = (hcol[:, :256] + w3[:, :256] / N)
    return out



# revision 2
# speedup vs baseline: 1.1779x; 1.1779x over previous
"""LinearSelfAttention kernel for TRN2 (8 NeuronCores, batch-parallel).

Key identity: with Hn = H[:, :n] (mask drops column n from the s-sum),
    attn = P H mask(H^T Q H) = C H,   C = P G Q,   G = Hn Hn^T  (257x257)
so  out = H + C H / n = H + Ct^T H / n,  Ct = C^T = Q^T G P^T.

Device computes ONLY the attention term A = Ct^T H (256-dim blocks); the
host adds H exactly in f32 plus thin rank-1 edge corrections (the e=256 /
a=256 slices of the chain, output row d=256, column t=2048).  This removes
every K=1 edge matmul AND the bf16 H input copy:

 - G via fp8e4m3 DoubleRow (K=256/pass), host-transposed Hn (fp8, 1.1MB).
 - Small chain V = G P^T, Ct = Q^T V in bf16 (K=128 passes, 256-blocks).
 - Ct evicted PSUM->SBUF as fp8 in DoubleRow weight layout; S4 A = Ct^T H
   runs fp8 DoubleRow with Ct stationary: 8 passes of N=512 per sample
   (vs 16 bf16 + 8 edge passes in the old scheme).  H arrives as a second
   fp8 copy in natural layout (1.05MB vs 2.1MB bf16).  Precision is fine
   because Ct ~ O(1) entries scale the fp8 H error by ~Ct*dH ~ 1e-4 of
   the output scale.
 - Input DMA: Ht split sync/scalar so G(0) starts ~3.5us in; qpi on
   gpsimd.  Y stores round-robin sync/gpsimd, fine-grained near the end
   so the post-compute drain is short.

Sharding: data-parallel over batch, 2 samples per core.
"""

import sys

sys.path.insert(0, "/opt/trn_rl_repo")

import numpy as np
import ml_dtypes

B, D1, N1 = 16, 257, 2049  # batch, d+1, n+1
N = N1 - 1  # 2048
NCORES = 8
BPC = B // NCORES  # samples per core

NT8 = N // 256  # 8 double-row s-tiles of the transposed Hn
DPAD = 272  # fp8 DR LDWEIGHTS: step between the 2 K-subtiles must be %16==0
HPAD = 2064  # padded t-dim of the natural-layout fp8 H (%16==0)
TCH = [(i * 512, 512) for i in range(4)]  # t=2048 column done on host
NWARM = 20

_cached = {}


def _build():
    import concourse.bass as bass
    import concourse.tile as tile
    from concourse import bacc, mybir
    from contextlib import ExitStack

    f32 = mybir.dt.float32
    bf16 = mybir.dt.bfloat16
    f8 = mybir.dt.float8e4
    DR = mybir.MatmulPerfMode.DoubleRow

    nc = bacc.Bacc("TRN2", target_bir_lowering=False, debug=False, num_devices=NCORES)

    Ht_d = nc.declare_dram_parameter("Ht", [BPC, NT8, 128, 2, DPAD], f8, isOutput=False)
    Hf_d = nc.declare_dram_parameter("Hf", [BPC, 128, 2, HPAD], f8, isOutput=False)
    QPI_d = nc.declare_dram_parameter("QPI", [256, 2 * D1], bf16, isOutput=False)
    Y_d = nc.declare_dram_parameter("Y", [BPC, 256, N], bf16, isOutput=True)

    with tile.TileContext(nc) as tc:
        with ExitStack() as ctx:
            const = ctx.enter_context(tc.tile_pool(name="const", bufs=1))
            htp = ctx.enter_context(tc.tile_pool(name="htp", bufs=2))
            hfp = ctx.enter_context(tc.tile_pool(name="hfp", bufs=2))
            sq = ctx.enter_context(tc.tile_pool(name="sq", bufs=2))
            yp = ctx.enter_context(tc.tile_pool(name="yp", bufs=2))

            # ---- input DMAs.  Ht (the G operand) first, split across the
            # two HWDGE queues so sample 0's tiles land ~3.5us in; qpi on
            # gpsimd (SWDGE) in parallel; the fp8 natural-layout H after.
            ht = [None] * BPC
            hf = [None] * BPC
            qpi = []
            for b in range(BPC):
                t = htp.tile([128, NT8, 2, DPAD], f8, tag="ht", name=f"ht{b}")
                if b == 0:
                    nc.sync.dma_start(t[:, 0:4, :, :], Ht_d[b, 0:4])
                    nc.scalar.dma_start(t[:, 4:8, :, :], Ht_d[b, 4:8])
                else:
                    nc.sync.dma_start(t[:, 4:8, :, :], Ht_d[b, 4:8])
                    nc.scalar.dma_start(t[:, 0:4, :, :], Ht_d[b, 0:4])
                ht[b] = t
            for c in range(2):
                t = const.tile([128, 2 * D1], bf16, tag=f"qpi{c}", name=f"qpi{c}")
                nc.gpsimd.dma_start(t[:, :], QPI_d[c * 128 : (c + 1) * 128, :])
                qpi.append(t)
            for b in range(BPC):
                t = hfp.tile([128, 2, HPAD], f8, tag="hf", name=f"hf{b}")
                eng = nc.sync if b == 0 else nc.scalar
                eng.dma_start(t[:, :, :N1], Hf_d[b, :, :, :N1])
                hf[b] = t

            # ---- PE warmup: ride the clock ramp until the first tile lands
            wsb = const.tile([128, 128], bf16, tag="wsb", name="wsb")
            nc.vector.memset(wsb[:, :], 0.0)
            with tc.tile_pool(name="wp", bufs=1, space="PSUM") as wp:
                wps = wp.tile([128, 512], f32, tag="wps", name="warm_ps")
                for i in range(NWARM):
                    nc.tensor.matmul(
                        wps[:, 0:128],
                        wsb[:, :],
                        wsb[:, :],
                        start=(i == 0),
                        stop=(i == NWARM - 1),
                    )

            with (
                tc.tile_pool(name="ppa", bufs=4, space="PSUM") as ppa,
                tc.tile_pool(name="ppb", bufs=4, space="PSUM") as ppb,
            ):
                gsb = [None] * BPC
                vsb = [None] * BPC
                ct8 = [None] * BPC

                def g_stage(b):
                    # ---- G = Hn Hn^T (fp8 DoubleRow, K=256 per pass)
                    gA = ppa.tile([128, 512], f32, tag="A", name=f"gA{b}")
                    gB = ppb.tile([128, 512], f32, tag="B", name=f"gB{b}")
                    regions = [gA[:, 0:D1], gB[:, 0:D1]]
                    for st in range(NT8):
                        for ac in range(2):
                            nc.tensor.matmul(
                                regions[ac][:, :],
                                ht[b][:, st, :, ac * 128 : (ac + 1) * 128],
                                ht[b][:, st, :, :D1],
                                start=(st == 0),
                                stop=(st == NT8 - 1),
                                perf_mode=DR,
                            )
                    gs = []
                    for ac in range(2):
                        t = sq.tile([128, D1], bf16, tag=f"g{ac}", name=f"gs{b}_{ac}")
                        eng = nc.scalar.copy if ac % 2 == 0 else nc.vector.tensor_copy
                        eng(t[:, :], regions[ac][:, :])
                        gs.append(t)
                    gsb[b] = gs

                def vct_stage(b):
                    # ---- V = G P^T  (G symmetric: lhsT slices G directly)
                    vA = ppa.tile([128, 512], f32, tag="A", name=f"vA{b}")
                    vB = ppb.tile([128, 512], f32, tag="B", name=f"vB{b}")
                    regions = [vA[:, 0:D1], vB[:, 0:D1]]
                    for kb in range(2):
                        for am in range(2):
                            nc.tensor.matmul(
                                regions[am][:, :],
                                gsb[b][kb][:, am * 128 : (am + 1) * 128],
                                qpi[kb][:, D1 : 2 * D1],
                                start=(kb == 0),
                                stop=(kb == 1),
                            )
                    vs = []
                    for am in range(2):
                        t = sq.tile([128, D1], bf16, tag=f"v{am}", name=f"vs{b}_{am}")
                        eng = nc.scalar.copy if am % 2 == 1 else nc.vector.tensor_copy
                        eng(t[:, :], regions[am][:, :])
                        vs.append(t)
                    vsb[b] = vs

                    # ---- Ct = Q^T V  (= C^T = n * Ct; host divides by n)
                    cA = ppa.tile([128, 512], f32, tag="A", name=f"cA{b}")
                    cB = ppb.tile([128, 512], f32, tag="B", name=f"cB{b}")
                    cregions = [cA[:, 0:D1], cB[:, 0:D1]]
                    for ka in range(2):
                        for em in range(2):
                            nc.tensor.matmul(
                                cregions[em][:, :],
                                qpi[ka][:, em * 128 : (em + 1) * 128],
                                vsb[b][ka][:, :],
                                start=(ka == 0),
                                stop=(ka == 1),
                            )
                    # evict as fp8 in the DoubleRow weight layout:
                    # ct8[p, i, d] = Ct[i*128+p, d]
                    t = sq.tile([128, 2, DPAD], f8, tag="ct8", name=f"ct8_{b}")
                    nc.scalar.copy(t[:, 0, 0:256], cregions[0][:, 0:256])
                    nc.vector.tensor_copy(t[:, 1, 0:256], cregions[1][:, 0:256])
                    ct8[b] = t

                def s4_stage(b):
                    # ---- A[d<256] = Ct^T H, fp8 DoubleRow, Ct stationary
                    y = [
                        yp.tile([128, N], bf16, tag=f"y{dc}", name=f"y{b}_{dc}")
                        for dc in range(2)
                    ]
                    pools = [ppa, ppb]
                    tags = ["A", "B"]
                    for dc in range(2):
                        for ti, (toff, tsz) in enumerate(TCH):
                            p = pools[dc].tile(
                                [128, 512], f32, tag=tags[dc], name=f"p{b}_{dc}_{ti}"
                            )
                            nc.tensor.matmul(
                                p[:128, :tsz],
                                ct8[b][:, :, dc * 128 : (dc + 1) * 128],
                                hf[b][:, :, toff : toff + tsz],
                                start=True,
                                stop=True,
                                perf_mode=DR,
                            )
                            eng = nc.scalar.copy if dc == 0 else nc.vector.tensor_copy
                            eng(y[dc][:, toff : toff + tsz], p[:128, :tsz])
                            # stores: eager, finer-grained for the last sample
                            # so the post-compute drain is short
                            q = nc.sync if dc == 0 else nc.gpsimd
                            dsl = slice(dc * 128, (dc + 1) * 128)
                            if b < BPC - 1:
                                if ti == 1:
                                    q.dma_start(Y_d[b, dsl, 0:1024], y[dc][:, 0:1024])
                                elif ti == 3:
                                    q.dma_start(Y_d[b, dsl, 1024:N], y[dc][:, 1024:N])
                            else:
                                if ti >= 1:
                                    sl = [
                                        None,
                                        slice(0, 1024),
                                        slice(1024, 1536),
                                        slice(1536, N),
                                    ][ti]
                                    q.dma_start(Y_d[b, dsl, sl], y[dc][:, sl])

                # emission order: G back-to-back, then chain(0) feeding
                # S4(0) early (output DMA starts sooner), chain(1)+S4(1)
                g_stage(0)
                g_stage(1)
                vct_stage(0)
                s4_stage(0)
                vct_stage(1)
                s4_stage(1)

    nc.compile()
    return nc


def _prep_in_maps(H, P, Q):
    bf = ml_dtypes.bfloat16
    f8 = ml_dtypes.float8_e4m3
    H = np.ascontiguousarray(H, dtype=np.float32)
    # G operand: [st, p, i, d] with s = st*256 + i*128 + p
    Ht = np.swapaxes(H[:, :, :N], 1, 2).reshape(B, NT8, 2, 128, D1)
    Ht8 = np.zeros((B, NT8, 128, 2, DPAD), dtype=f8)
    Ht8[..., :D1] = np.swapaxes(Ht, 2, 3).astype(f8)
    # S4 operand: natural layout fp8, [p, c, t] = H[c*128+p, t]
    Hf8 = np.zeros((B, 128, 2, HPAD), dtype=f8)
    Hf8[..., :N1] = np.swapaxes(H[:, :256, :].reshape(B, 2, 128, N1), 1, 2).astype(f8)
    QPI = np.ascontiguousarray(
        np.concatenate([Q, P.T], axis=1)[:256].astype(bf)
    )
    return [
        {
            "Ht": Ht8[c * BPC : (c + 1) * BPC],
            "Hf": Hf8[c * BPC : (c + 1) * BPC],
            "QPI": QPI,
        }
        for c in range(NCORES)
    ]


def kernel(H, P, Q):
    from concourse.bass_utils import run_bass_kernel_spmd

    if "nc" not in _cached:
        _cached["nc"] = _build()
    nc = _cached["nc"]

    in_maps = _prep_in_maps(H, P, Q)
    for attempt in range(3):
        res = run_bass_kernel_spmd(nc, in_maps, list(range(NCORES)))
        adev = np.concatenate(
            [res.results[c]["Y"].astype(np.float32) for c in range(NCORES)], axis=0
        )
        if np.isfinite(adev).all():
            break

    H = np.ascontiguousarray(H, dtype=np.float32)
    P = np.ascontiguousarray(P, dtype=np.float32)
    Q = np.ascontiguousarray(Q, dtype=np.float32)
    Hn = H[:, :, :N]
    Hm = Hn[:, :256, :]  # rows 0..255

    out = np.empty((B, D1, N1), dtype=np.float32)
    # device part: A over (m<256, e<256) chain blocks
    out[:, :256, :N] = H[:, :256, :N] + adev / N

    # ---- host edge corrections (all thin O(n d) rank-1 terms, exact f32)
    # g256[e] = G[256, e]; v256[d] = V[256, d]; er256[d] = C[d,256]/n
    g256 = np.einsum("bds,bs->bd", Hn, Hn[:, 256, :])
    v256 = g256 @ P.T
    t1 = np.einsum("bds,d->bs", Hn, Q[:, 256])
    r = np.einsum("bds,bs->bd", Hn, t1)  # = G @ Q[:,256]
    er256 = (r @ P.T) / N
    # m=256 term of A: er256 (x) H[256, :]
    out[:, :256, :N] += er256[:, :256, None] * Hn[:, None, 256, :]
    # a=256 / e=256 chain slices for m<256:
    #   corr[m,d] = P[d,256] * (sum_a<256 Q[a,m] g256[a])/n + Q[256,m] v256[d]/n
    qg = np.einsum("ba,am->bm", g256[:, :256], Q[:256, :256])
    z1 = np.einsum("bm,bmt->bt", qg, Hm) / N
    z2 = np.einsum("m,bmt->bt", Q[256, :256], Hm) / N
    out[:, :256, :N] += P[None, :256, 256, None] * z1[:, None, :]
    out[:, :256, :N] += v256[:, :256, None] * z2[:, None, :]

    # output row d=256 exactly, on host (fp32)
    u = np.einsum("bds,d->bs", Hn, P[256, :])
    v = np.einsum("bds,bs->bd", Hn, u)  # = G @ P[256,:] per sample
    c256 = v @ Q  # = C[256, :] per sample
    out[:, 256, :] = H[:, 256, :] + np.einsum("bd,bdt->bt", c256, H) / N
    # exact column t=2048 on host: C @ hcol = P (G (Q hcol))
    hcol = H[:, :, N]  # (B, 257)
    w1 = hcol @ Q.T  # (Q hcol)[a]
    w2 = np.einsum("bds,bs->bd", Hn, np.einsum("bds,bd->bs", Hn, w1))  # G w1
    w3 = w2 @ P.T
    out[:, :256, N] = hcol[:, :256] + w3[:, :256] / N
    return out
